# revision 26
# baseline (speedup 1.0000x reference)
"""DGL capsule routing layer on 8 trn2 NeuronCores (Bass/Tile).

Math: for routing_num iterations,
    c = softmax(b, axis=out)                        # b0 = 0
    s = einsum('io,iof->of', c, uh)
    v = squash(s)
    b = b + einsum('iof,of->io', uh, v)
Output: final v [OUT, F].

Key identity: b_t = uh . (v_1 + ... + v_{t-1}) (b is linear in uh), so each
iteration is one pass over uh with w_t = cumulative sum of v's:
    pass t: b = sum_f uh[i,o,f]*w[o,f]; e = exp(b); r_i = 1/sum_o e
            s[o,f] = sum_i r_i * e[i,o] * uh[i,o,f]   (partial per core)
            AllReduce(s); v = squash(s); w += v
Pass 1 has c uniform (=1/OUT) so it is a pure PE pass.

Performance structure (vs the f32 streaming baseline):
  - u_hat is cast to bf16 AND transposed to [i, f, o] on the host. With o
    contiguous innermost, every big DVE elementwise op has both operands
    16-bit/step-1/4B-aligned, which is exactly the condition for the DVE's
    2x_1P packed mode (2 elem/cycle/lane) -- f32 or broadcast-stride
    operands would pin them to 1x.
  - 3 of 4 i-blocks are SBUF-resident bf16 (loaded once in pass 1); the 4th
    streams from HBM each pass (DMA is otherwise idle in passes >= 2).
  - b = sum_f uh*w is a contiguous binary tree of TT adds (all 2x) instead
    of a 1x segmented reduce; p = e*uh is issued per-f as fully contiguous
    [P,1024] TTs (2x) instead of a broadcast TT (1x).
  - s-partials for block pairs accumulate in one PSUM tile, halving the
    1-lane PSUM->SBUF flushes; the AllReduce runs in bf16 (2x16384 = 64KiB).
  - GpSimd (slow ucode TT, ~60 G elem/s) takes a minority share of the tm
    muls; ACT does exp + psum flushes; PE does the rank-1 s matmuls.

Data layouts: SBUF/DRAM s/w vectors are (f,o)-flat ("T" layout, o fastest);
the post-AllReduce load and the w_dram store use mirrored strided APs to
convert to/from the o-major [128, (j f)] spread (o = p*8+j) used by squash.
"""

import numpy as np
import ml_dtypes
from contextlib import ExitStack

import concourse.bass as bass
import concourse.mybir as mybir
import concourse.tile as tile
from concourse import bacc
from concourse import bass_utils

F32 = mybir.dt.float32
BF16 = mybir.dt.bfloat16
AX = mybir.AxisListType
AF = mybir.ActivationFunctionType

IN_NODES, OUT_NODES, F_SIZE = 4096, 1024, 16
CORES = 8
I_LOC = IN_NODES // CORES          # 512 in-nodes per core
ROW = OUT_NODES * F_SIZE           # 16384 elems per in-node row
P = 128
NBLK = I_LOC // P                  # 4 i-blocks per core
NRES = 3                           # blocks 0..2 SBUF-resident; block 3 streamed
NPAIR = NBLK // 2                  # 2 block pairs
CH = 4096                          # tm chunk width (4 f-slices x 1024 o)
NCH = ROW // CH                    # 4 tm chunks per block
SEG = 2048                         # psum segment width (f-pair x 1024 o)
NSEG = ROW // SEG                  # 8 segments per block
MMW = 512                          # matmul moving width
O2 = OUT_NODES                     # 1024
GP_TM = set()                      # tm chunks on GpSimd: its ucode TT is ~4x
                                   # slower AND stalls DVE via the shared
                                   # SBUF port -- keep everything on DVE
N_WARM_MM = 24                     # junk matmuls to lift the PE HAM throttle


def _body(nc, tc, uh, v_out, R, rg):
    uh_t = uh.rearrange("(n p) r -> n p r", p=P)   # [NBLK, 128, 16384] bf16

    with ExitStack() as ctx:
        persist = ctx.enter_context(tc.tile_pool(name="persist", bufs=1))
        tmpool = ctx.enter_context(tc.tile_pool(name="tmpool", bufs=2))
        t8pool = ctx.enter_context(tc.tile_pool(name="t8pool", bufs=2))
        ppool = ctx.enter_context(tc.tile_pool(name="ppool", bufs=2))
        spool = ctx.enter_context(tc.tile_pool(name="spool", bufs=2))
        small = ctx.enter_context(tc.tile_pool(name="small", bufs=2))
        pspool = ctx.enter_context(tc.tile_pool(name="pspool", bufs=2, space="PSUM"))
        dram = ctx.enter_context(tc.tile_pool(name="dram", bufs=2, space="DRAM"))

        c0 = persist.tile([P, 1], BF16, name="c0")
        nc.vector.memset(c0, 1.0 / OUT_NODES)
        uh_sb = [persist.tile([P, ROW], BF16, name=f"uh{b}") for b in range(NRES)]
        w_sb = w_acc = None
        if R > 1:
            w_sb = persist.tile([P, ROW], BF16, name="w_sb")
            w_acc = persist.tile([P, P], F32, name="w_acc")

        # tiny dummy AllReduce: pays the ~60us first-collective setup cost
        # concurrently with pass 1 instead of on the pass-1 -> pass-2 boundary
        warm_in = dram.tile([16], BF16, tag="warm_in")
        warm_out = dram.tile([16], BF16, tag="warm_out")
        nc.gpsimd.collective_compute(
            "AllReduce", mybir.AluOpType.add, replica_groups=rg,
            ins=[warm_in.opt()], outs=[warm_out.opt()],
        )
        # back-to-back junk matmuls: ~3.4us of sustained PE busy flips the
        # HAM clock gate from 1.2 to 2.4 GHz before the real matmuls start
        junk = persist.tile([P, MMW], BF16, name="junk")
        nc.vector.memset(junk, 0.0)
        ps_warm = pspool.tile([1, SEG], F32, tag="ps")
        for _ in range(N_WARM_MM):
            nc.tensor.matmul(ps_warm[:, :MMW], c0, junk,
                             start=True, stop=True, skip_group_check=True)

        # resident uh upload happens inside the pass-1 segment loop, one
        # [P, SEG] column-slice per block per segment, so each psum group
        # closes as soon as its own ~2 MB lands (sub-tile deps) instead of
        # stalling on whole-block loads

        def stream_chunk(lo, width):
            st = spool.tile([P, width], BF16, tag="st")
            nc.sync.dma_start(st, uh_t[NBLK - 1, :, lo:lo + width])
            return st

        def warm_mm(n=2):
            # keep the PE HAM window busy so phase-B matmuls run at 2.4 GHz
            pw = pspool.tile([1, SEG], F32, tag="ps")
            for _ in range(n):
                nc.tensor.matmul(pw[:, :MMW], c0, junk,
                                 start=True, stop=True, skip_group_check=True)

        w_dram = None
        for t in range(1, R + 1):
            ar_in = dram.tile([1 if t == 1 else NPAIR, ROW], BF16,
                              tag="ar1" if t == 1 else "ar_in")
            if t > 1:
                # broadcast w_T (written at the end of pass t-1) to all
                # 128 partitions, chunk by chunk
                wd_b = w_dram.unsqueeze(0)
                for c in range(NSEG):
                    sl = slice(c * SEG, (c + 1) * SEG)
                    nc.sync.dma_start(
                        w_sb[:, sl], wd_b[:, sl].broadcast_to([P, SEG]))
            if t == 1:
                # c is uniform: pure PE pass, all 4 blocks accumulate into
                # one psum tile per segment (half the flushes, half the AR)
                for s in range(NSEG):
                    ss = slice(s * SEG, (s + 1) * SEG)
                    if s % 2 == 0:
                        src2 = stream_chunk(s * SEG, 2 * SEG)
                    for blk in range(NRES):
                        nc.sync.dma_start(uh_sb[blk][:, ss],
                                          uh_t[blk, :, ss])
                    ps = pspool.tile([1, SEG], F32, tag="ps")
                    for blk in range(NBLK):
                        if blk < NRES:
                            src = uh_sb[blk][:, ss]
                        elif s % 2 == 0:
                            src = src2[:, :SEG]
                        else:
                            src = src2[:, SEG:]
                        for cix in range(SEG // MMW):
                            msl = slice(cix * MMW, (cix + 1) * MMW)
                            nc.tensor.matmul(
                                ps[:, msl], c0, src[:, msl],
                                start=(blk == 0), stop=(blk == NBLK - 1),
                                skip_group_check=True,
                            )
                    fl = small.tile([1, SEG], BF16, tag="fl")
                    # DVE is idle in pass 1: alternate flush engines so the
                    # PE is never flush-gated (keeps the HAM clock warm too)
                    if s % 2 == 0:
                        nc.scalar.copy(fl, ps)
                    else:
                        nc.vector.tensor_copy(fl, ps)
                    nc.sync.dma_start(ar_in[0, ss], fl)
            for pair in range(NPAIR if t > 1 else 0):
                es, rinvs = [None, None], [None, None]
                if t > 1:
                    # phase A: b = sum_f uh*w via tm mul + tree adds
                    for j in range(2):
                        blk = 2 * pair + j
                        tms = []
                        for c in range(NCH):
                            sl = slice(c * CH, (c + 1) * CH)
                            if blk < NRES:
                                src = uh_sb[blk][:, sl]
                            else:
                                src = stream_chunk(c * CH, CH)
                            tm = tmpool.tile([P, CH], BF16, tag="tm")
                            eng = nc.gpsimd if c in GP_TM else nc.vector
                            eng.tensor_mul(tm, src, w_sb[:, sl])
                            tms.append(tm)
                        t8a = t8pool.tile([P, CH], BF16, tag="t8")
                        nc.vector.tensor_add(t8a, tms[0], tms[1])
                        t8b = t8pool.tile([P, CH], BF16, tag="t8")
                        nc.vector.tensor_add(t8b, tms[2], tms[3])
                        nc.vector.tensor_add(t8a, t8a, t8b)
                        nc.vector.tensor_add(
                            t8a[:, :SEG], t8a[:, :SEG], t8a[:, SEG:])
                        b_t = small.tile([P, O2], BF16, tag="b")
                        nc.vector.tensor_add(
                            b_t, t8a[:, :O2], t8a[:, O2:SEG])
                        e_t = small.tile([P, O2], BF16, tag="e", bufs=3)
                        den = small.tile([P, 1], F32, tag="den", bufs=4)
                        nc.scalar.activation(e_t, b_t, AF.Exp, accum_out=den)
                        rinv_f = small.tile([P, 1], F32, tag="rinv_f", bufs=4)
                        nc.vector.reciprocal(rinv_f, den)
                        rinv_b = small.tile([P, 1], BF16, tag="rinv", bufs=4)
                        nc.vector.tensor_copy(rinv_b, rinv_f)
                        es[j], rinvs[j] = e_t, rinv_b
                        warm_mm()
                # phase B: p = e*uh per f-slice; psum[1,SEG] += rinv . p
                for s in range(NSEG):
                    ss = slice(s * SEG, (s + 1) * SEG)
                    ps = pspool.tile([1, SEG], F32, tag="ps")
                    for j in range(2):
                        blk = 2 * pair + j
                        if blk < NRES:
                            src = uh_sb[blk][:, ss]
                        elif s % 2 == 0:
                            src2 = stream_chunk(s * SEG, 2 * SEG)
                            src = src2[:, :SEG]
                        else:
                            src = src2[:, SEG:]
                        pp = ppool.tile([P, SEG], BF16, tag="pp")
                        nc.vector.tensor_mul(
                            pp.rearrange("p (g o) -> p g o", g=2),
                            src.rearrange("p (g o) -> p g o", g=2),
                            es[j][:, None, :].broadcast_to([P, 2, O2]))
                        mv, stat = pp, rinvs[j]
                        for cix in range(SEG // MMW):
                            msl = slice(cix * MMW, (cix + 1) * MMW)
                            nc.tensor.matmul(
                                ps[:, msl], stat, mv[:, msl],
                                start=(j == 0), stop=(j == 1),
                                skip_group_check=True,
                            )
                    fl = small.tile([1, SEG], BF16, tag="fl")
                    nc.scalar.copy(fl, ps)
                    nc.sync.dma_start(ar_in[pair, ss], fl)
            ar_out = dram.tile([1 if t == 1 else NPAIR, ROW], BF16,
                               tag="ar1o" if t == 1 else "ar_out")
            nc.gpsimd.collective_compute(
                "AllReduce", mybir.AluOpType.add, replica_groups=rg,
                ins=[ar_in.opt()], outs=[ar_out.opt()],
            )
            # XBAR-transposed load: sld[o_lo, (f,oh)] = s_T[f, oh*128+o_lo]
            slds = []
            for r in range(1 if t == 1 else NPAIR):
                sld = small.tile([P, P], BF16, tag="sld")
                nc.sync.dma_start(
                    sld, ar_out[r].rearrange("(a b) -> a b", b=P),
                    transpose=True)
                slds.append(sld)
            s2 = small.tile([P, P], F32, tag="s2", bufs=1)
            if t == 1:
                nc.vector.tensor_copy(s2, slds[0])
            else:
                nc.vector.tensor_add(s2, slds[0], slds[1])
            # squash: v = s * sqrt(sq)/(1+sq), sq = sum_f s^2
            # layout here is [o_lo, (f, oh)]: f has stride 8, oh stride 1
            ssq = small.tile([P, P], F32, tag="ssq", bufs=1)
            nc.vector.tensor_mul(ssq, s2, s2)
            sq = small.tile([P, 8], F32, tag="sq", bufs=1)
            nc.vector.reduce_sum(
                sq, ssq.rearrange("p (f oh) -> p oh f", oh=8), axis=AX.X)
            # sqrt via exp(0.5*ln(x)): stays in the exp/ln ACT table set
            lnq = small.tile([P, 8], F32, tag="lnq", bufs=1)
            nc.scalar.activation(lnq, sq, AF.Ln)
            y = small.tile([P, 8], F32, tag="y", bufs=1)
            nc.scalar.activation(y, lnq, AF.Exp, scale=0.5)
            d1 = small.tile([P, 8], F32, tag="d1", bufs=1)
            nc.vector.tensor_scalar_add(d1, sq, 1.0)
            rd = small.tile([P, 8], F32, tag="rd", bufs=1)
            nc.vector.reciprocal(rd, d1)
            sc = small.tile([P, 8], F32, tag="sc", bufs=1)
            nc.vector.tensor_mul(sc, y, rd)
            v_sb = small.tile([P, P], F32, tag="v_sb", bufs=1)
            nc.vector.tensor_mul(
                v_sb.rearrange("p (f oh) -> p f oh", oh=8),
                s2.rearrange("p (f oh) -> p f oh", oh=8),
                sc[:, None, :].broadcast_to([P, F_SIZE, 8]),
            )
            if t == R:
                # v_sb[o_lo,(f,oh)] -> transpose -> [(f,oh), o_lo] which is
                # exactly T-flat (f*1024+o); host un-transposes for free
                v_bf = small.tile([P, P], BF16, tag="v_bf", bufs=1)
                nc.vector.tensor_copy(v_bf, v_sb)
                v_ts = small.tile([P, P], BF16, tag="v_ts", bufs=1)
                nc.sync.dma_start(v_ts, v_bf, transpose=True)
                nc.sync.dma_start(
                    v_out.rearrange("f (oh ol) -> (f oh) ol", ol=P), v_ts)
            else:
                if t == 1:
                    nc.scalar.copy(w_acc, v_sb)
                else:
                    nc.vector.tensor_add(w_acc, w_acc, v_sb)
                w_bf = small.tile([P, P], BF16, tag="w_bf", bufs=1)
                nc.vector.tensor_copy(w_bf, w_acc)
                w_ts = small.tile([P, P], BF16, tag="w_ts", bufs=1)
                nc.sync.dma_start(w_ts, w_bf, transpose=True)
                w_dram = dram.tile([ROW], BF16, tag="w_dram")
                nc.sync.dma_start(
                    w_dram.rearrange("(a b) -> a b", b=P), w_ts)


def _build(routing_num: int):
    R = int(routing_num)
    assert R >= 1
    nc = bacc.Bacc(
        "TRN2", target_bir_lowering=False, debug=False, num_devices=CORES)
    uh = nc.dram_tensor("uh", [I_LOC, ROW], BF16, kind="ExternalInput")
    v_out = nc.dram_tensor("v_out", [F_SIZE, OUT_NODES], BF16,
                           kind="ExternalOutput")
    rg = [list(range(CORES))]
    with tile.TileContext(nc) as tc:
        _body(nc, tc, uh.ap(), v_out.ap(), R, rg)
    nc.compile()
    return nc


_CACHE: dict = {}


def _get_nc(routing_num: int):
    R = int(routing_num)
    if R not in _CACHE:
        _CACHE[R] = _build(R)
    return _CACHE[R]


def _shard(u_hat: np.ndarray):
    uh = np.asarray(u_hat, dtype=np.float32)
    assert uh.shape == (IN_NODES * OUT_NODES, F_SIZE), uh.shape
    # [i, o, f] -> [i, f, o] ("T" layout, o contiguous innermost), bf16
    uh = uh.reshape(IN_NODES, OUT_NODES, F_SIZE).transpose(0, 2, 1)
    uh = np.ascontiguousarray(uh, dtype=ml_dtypes.bfloat16)
    uh = uh.reshape(IN_NODES, ROW)
    return [{"uh": uh[k * I_LOC:(k + 1) * I_LOC]} for k in range(CORES)]


def run(u_hat, routing_num, trace=False):
    nc = _get_nc(routing_num)
    in_maps = _shard(u_hat)
    res = bass_utils.run_bass_kernel_spmd(
        nc, in_maps, core_ids=list(range(CORES)), trace=trace)
    return res


def kernel(u_hat, routing_num):
    res = run(u_hat, routing_num, trace=False)
    # device returns v in [f, o] (T) layout bf16; un-transpose on host
    v_t = np.asarray(res.results[0]["v_out"], dtype=np.float32)
    return np.ascontiguousarray(v_t.reshape(F_SIZE, OUT_NODES).T)


# revision 30
# speedup vs baseline: 1.0145x; 1.0145x over previous
"""DGL capsule routing layer on 8 trn2 NeuronCores (Bass/Tile).

Math: for routing_num iterations,
    c = softmax(b, axis=out)                        # b0 = 0
    s = einsum('io,iof->of', c, uh)
    v = squash(s)
    b = b + einsum('iof,of->io', uh, v)
Output: final v [OUT, F].

Key identity: b_t = uh . (v_1 + ... + v_{t-1}) (b is linear in uh), so each
iteration is one pass over uh with w_t = cumulative sum of v's:
    pass t: b = sum_f uh[i,o,f]*w[o,f]; e = exp(b); r_i = 1/sum_o e
            s[o,f] = sum_i r_i * e[i,o] * uh[i,o,f]   (partial per core)
            AllReduce(s); v = squash(s); w += v
Pass 1 has c uniform (=1/OUT) so it is a pure PE pass.

Performance structure (vs the f32 streaming baseline):
  - u_hat is cast to bf16 AND transposed to [i, f, o] on the host. With o
    contiguous innermost, every big DVE elementwise op has both operands
    16-bit/step-1/4B-aligned, which is exactly the condition for the DVE's
    2x_1P packed mode (2 elem/cycle/lane) -- f32 or broadcast-stride
    operands would pin them to 1x.
  - 3 of 4 i-blocks are SBUF-resident bf16 (loaded once in pass 1); the 4th
    streams from HBM each pass (DMA is otherwise idle in passes >= 2).
  - b = sum_f uh*w is a contiguous binary tree of TT adds (all 2x) instead
    of a 1x segmented reduce; p = e*uh is issued per-f as fully contiguous
    [P,1024] TTs (2x) instead of a broadcast TT (1x).
  - s-partials for block pairs accumulate in one PSUM tile, halving the
    1-lane PSUM->SBUF flushes; the AllReduce runs in bf16 (2x16384 = 64KiB).
  - GpSimd (slow ucode TT, ~60 G elem/s) takes a minority share of the tm
    muls; ACT does exp + psum flushes; PE does the rank-1 s matmuls.

Data layouts: SBUF/DRAM s/w vectors are (f,o)-flat ("T" layout, o fastest);
the post-AllReduce load and the w_dram store use mirrored strided APs to
convert to/from the o-major [128, (j f)] spread (o = p*8+j) used by squash.
"""

import numpy as np
import ml_dtypes
from contextlib import ExitStack

import concourse.bass as bass
import concourse.mybir as mybir
import concourse.tile as tile
from concourse import bacc
from concourse import bass_utils

F32 = mybir.dt.float32
BF16 = mybir.dt.bfloat16
AX = mybir.AxisListType
AF = mybir.ActivationFunctionType

IN_NODES, OUT_NODES, F_SIZE = 4096, 1024, 16
CORES = 8
I_LOC = IN_NODES // CORES          # 512 in-nodes per core
ROW = OUT_NODES * F_SIZE           # 16384 elems per in-node row
P = 128
NBLK = I_LOC // P                  # 4 i-blocks per core
NRES = 3                           # blocks 0..2 SBUF-resident; block 3 streamed
NPAIR = NBLK // 2                  # 2 block pairs
CH = 4096                          # tm chunk width (4 f-slices x 1024 o)
NCH = ROW // CH                    # 4 tm chunks per block
SEG = 2048                         # psum segment width (f-pair x 1024 o)
NSEG = ROW // SEG                  # 8 segments per block
MMW = 512                          # matmul moving width
O2 = OUT_NODES                     # 1024
GP_TM = set()                      # tm chunks on GpSimd: its ucode TT is ~4x
                                   # slower AND stalls DVE via the shared
                                   # SBUF port -- keep everything on DVE
N_WARM_MM = 24                     # junk matmuls to lift the PE HAM throttle


def _body(nc, tc, uh, v_out, R, rg):
    uh_t = uh.rearrange("(n p) r -> n p r", p=P)   # [NBLK, 128, 16384] bf16

    with ExitStack() as ctx:
        persist = ctx.enter_context(tc.tile_pool(name="persist", bufs=1))
        tmpool = ctx.enter_context(tc.tile_pool(name="tmpool", bufs=2))
        t8pool = ctx.enter_context(tc.tile_pool(name="t8pool", bufs=2))
        ppool = ctx.enter_context(tc.tile_pool(name="ppool", bufs=2))
        spool = ctx.enter_context(tc.tile_pool(name="spool", bufs=2))
        small = ctx.enter_context(tc.tile_pool(name="small", bufs=2))
        pspool = ctx.enter_context(tc.tile_pool(name="pspool", bufs=2, space="PSUM"))
        dram = ctx.enter_context(tc.tile_pool(name="dram", bufs=2, space="DRAM"))

        c0 = persist.tile([P, 1], BF16, name="c0")
        nc.vector.memset(c0, 1.0 / OUT_NODES)
        uh_sb = [persist.tile([P, ROW], BF16, name=f"uh{b}") for b in range(NRES)]
        w_sb = w_acc = None
        if R > 1:
            # one tile per CH chunk so a tm mul only waits on its own
            # chunk's broadcast, not the whole-w tile-level dependency
            w_sb = [persist.tile([P, CH], BF16, name=f"w{c}")
                    for c in range(NCH)]
            w_acc = persist.tile([P, P], F32, name="w_acc")

        # tiny dummy AllReduce: pays the ~60us first-collective setup cost
        # concurrently with pass 1 instead of on the pass-1 -> pass-2 boundary
        warm_in = dram.tile([16], BF16, tag="warm_in")
        warm_out = dram.tile([16], BF16, tag="warm_out")
        nc.gpsimd.collective_compute(
            "AllReduce", mybir.AluOpType.add, replica_groups=rg,
            ins=[warm_in.opt()], outs=[warm_out.opt()],
        )
        # back-to-back junk matmuls: ~3.4us of sustained PE busy flips the
        # HAM clock gate from 1.2 to 2.4 GHz before the real matmuls start
        junk = persist.tile([P, MMW], BF16, name="junk")
        nc.vector.memset(junk, 0.0)
        ps_warm = pspool.tile([1, SEG], F32, tag="ps")
        for _ in range(N_WARM_MM):
            nc.tensor.matmul(ps_warm[:, :MMW], c0, junk,
                             start=True, stop=True, skip_group_check=True)

        # resident uh upload happens inside the pass-1 segment loop, one
        # [P, SEG] column-slice per block per segment, so each psum group
        # closes as soon as its own ~2 MB lands (sub-tile deps) instead of
        # stalling on whole-block loads

        def stream_chunk(lo, width):
            st = spool.tile([P, width], BF16, tag="st")
            nc.sync.dma_start(st, uh_t[NBLK - 1, :, lo:lo + width])
            return st

        def warm_mm(n=2):
            # keep the PE HAM window busy so phase-B matmuls run at 2.4 GHz
            pw = pspool.tile([1, SEG], F32, tag="ps")
            for _ in range(n):
                nc.tensor.matmul(pw[:, :MMW], c0, junk,
                                 start=True, stop=True, skip_group_check=True)

        w_dram = None
        for t in range(1, R + 1):
            ar_in = dram.tile([1 if t == 1 else NPAIR, ROW], BF16,
                              tag="ar1" if t == 1 else "ar_in")
            if t > 1:
                # broadcast w_T (written at the end of pass t-1) to all
                # 128 partitions, chunk by chunk
                wd_b = w_dram.unsqueeze(0)
                for c in range(NCH):
                    sl = slice(c * CH, (c + 1) * CH)
                    nc.sync.dma_start(
                        w_sb[c], wd_b[:, sl].broadcast_to([P, CH]))
            if t == 1:
                # c is uniform: pure PE pass, all 4 blocks accumulate into
                # one psum tile per segment (half the flushes, half the AR)
                for s in range(NSEG):
                    ss = slice(s * SEG, (s + 1) * SEG)
                    if s % 2 == 0:
                        src2 = stream_chunk(s * SEG, 2 * SEG)
                    for blk in range(NRES):
                        # alternate the two HWDGE queues: transfers from one
                        # ring drain serially, two rings run in parallel
                        eng = nc.sync if (s + blk) % 2 == 0 else nc.scalar
                        eng.dma_start(uh_sb[blk][:, ss], uh_t[blk, :, ss])
                    ps = pspool.tile([1, SEG], F32, tag="ps")
                    for blk in range(NBLK):
                        if blk < NRES:
                            src = uh_sb[blk][:, ss]
                        elif s % 2 == 0:
                            src = src2[:, :SEG]
                        else:
                            src = src2[:, SEG:]
                        for cix in range(SEG // MMW):
                            msl = slice(cix * MMW, (cix + 1) * MMW)
                            nc.tensor.matmul(
                                ps[:, msl], c0, src[:, msl],
                                start=(blk == 0), stop=(blk == NBLK - 1),
                                skip_group_check=True,
                            )
                    fl = small.tile([1, SEG], BF16, tag="fl")
                    # DVE is idle in pass 1: alternate flush engines so the
                    # PE is never flush-gated (keeps the HAM clock warm too)
                    if s % 2 == 0:
                        nc.scalar.copy(fl, ps)
                    else:
                        nc.vector.tensor_copy(fl, ps)
                    nc.sync.dma_start(ar_in[0, ss], fl)
            for pair in range(NPAIR if t > 1 else 0):
                es, rinvs = [None, None], [None, None]
                if t > 1:
                    # phase A: b = sum_f uh*w via tm mul + tree adds
                    for j in range(2):
                        blk = 2 * pair + j
                        tms = []
                        for c in range(NCH):
                            sl = slice(c * CH, (c + 1) * CH)
                            if blk < NRES:
                                src = uh_sb[blk][:, sl]
                            else:
                                src = stream_chunk(c * CH, CH)
                            tm = tmpool.tile([P, CH], BF16, tag="tm")
                            eng = nc.gpsimd if c in GP_TM else nc.vector
                            eng.tensor_mul(tm, src, w_sb[c])
                            tms.append(tm)
                        t8a = t8pool.tile([P, CH], BF16, tag="t8")
                        nc.vector.tensor_add(t8a, tms[0], tms[1])
                        t8b = t8pool.tile([P, CH], BF16, tag="t8")
                        nc.vector.tensor_add(t8b, tms[2], tms[3])
                        nc.vector.tensor_add(t8a, t8a, t8b)
                        nc.vector.tensor_add(
                            t8a[:, :SEG], t8a[:, :SEG], t8a[:, SEG:])
                        b_t = small.tile([P, O2], BF16, tag="b")
                        nc.vector.tensor_add(
                            b_t, t8a[:, :O2], t8a[:, O2:SEG])
                        e_t = small.tile([P, O2], BF16, tag="e", bufs=3)
                        den = small.tile([P, 1], F32, tag="den", bufs=4)
                        nc.scalar.activation(e_t, b_t, AF.Exp, accum_out=den)
                        rinv_f = small.tile([P, 1], F32, tag="rinv_f", bufs=4)
                        nc.vector.reciprocal(rinv_f, den)
                        rinv_b = small.tile([P, 1], BF16, tag="rinv", bufs=4)
                        nc.vector.tensor_copy(rinv_b, rinv_f)
                        es[j], rinvs[j] = e_t, rinv_b
                        warm_mm()
                # phase B: p = e*uh per f-slice; psum[1,SEG] += rinv . p
                for s in range(NSEG):
                    ss = slice(s * SEG, (s + 1) * SEG)
                    ps = pspool.tile([1, SEG], F32, tag="ps")
                    for j in range(2):
                        blk = 2 * pair + j
                        if blk < NRES:
                            src = uh_sb[blk][:, ss]
                        elif s % 2 == 0:
                            src2 = stream_chunk(s * SEG, 2 * SEG)
                            src = src2[:, :SEG]
                        else:
                            src = src2[:, SEG:]
                        pp = ppool.tile([P, SEG], BF16, tag="pp")
                        nc.vector.tensor_mul(
                            pp.rearrange("p (g o) -> p g o", g=2),
                            src.rearrange("p (g o) -> p g o", g=2),
                            es[j][:, None, :].broadcast_to([P, 2, O2]))
                        mv, stat = pp, rinvs[j]
                        for cix in range(SEG // MMW):
                            msl = slice(cix * MMW, (cix + 1) * MMW)
                            nc.tensor.matmul(
                                ps[:, msl], stat, mv[:, msl],
                                start=(j == 0), stop=(j == 1),
                                skip_group_check=True,
                            )
                    fl = small.tile([1, SEG], BF16, tag="fl")
                    nc.scalar.copy(fl, ps)
                    nc.sync.dma_start(ar_in[pair, ss], fl)
            ar_out = dram.tile([1 if t == 1 else NPAIR, ROW], BF16,
                               tag="ar1o" if t == 1 else "ar_out")
            nc.gpsimd.collective_compute(
                "AllReduce", mybir.AluOpType.add, replica_groups=rg,
                ins=[ar_in.opt()], outs=[ar_out.opt()],
            )
            # XBAR-transposed load: sld[o_lo, (f,oh)] = s_T[f, oh*128+o_lo]
            slds = []
            for r in range(1 if t == 1 else NPAIR):
                sld = small.tile([P, P], BF16, tag="sld")
                nc.sync.dma_start(
                    sld, ar_out[r].rearrange("(a b) -> a b", b=P),
                    transpose=True)
                slds.append(sld)
            s2 = small.tile([P, P], F32, tag="s2", bufs=1)
            if t == 1:
                nc.vector.tensor_copy(s2, slds[0])
            else:
                nc.vector.tensor_add(s2, slds[0], slds[1])
            # squash: v = s * sqrt(sq)/(1+sq), sq = sum_f s^2
            # layout here is [o_lo, (f, oh)]: f has stride 8, oh stride 1
            ssq = small.tile([P, P], F32, tag="ssq", bufs=1)
            nc.vector.tensor_mul(ssq, s2, s2)
            sq = small.tile([P, 8], F32, tag="sq", bufs=1)
            nc.vector.reduce_sum(
                sq, ssq.rearrange("p (f oh) -> p oh f", oh=8), axis=AX.X)
            # sqrt via exp(0.5*ln(x)): stays in the exp/ln ACT table set
            lnq = small.tile([P, 8], F32, tag="lnq", bufs=1)
            nc.scalar.activation(lnq, sq, AF.Ln)
            y = small.tile([P, 8], F32, tag="y", bufs=1)
            nc.scalar.activation(y, lnq, AF.Exp, scale=0.5)
            d1 = small.tile([P, 8], F32, tag="d1", bufs=1)
            nc.vector.tensor_scalar_add(d1, sq, 1.0)
            rd = small.tile([P, 8], F32, tag="rd", bufs=1)
            nc.vector.reciprocal(rd, d1)
            sc = small.tile([P, 8], F32, tag="sc", bufs=1)
            nc.vector.tensor_mul(sc, y, rd)
            v_sb = small.tile([P, P], F32, tag="v_sb", bufs=1)
            nc.vector.tensor_mul(
                v_sb.rearrange("p (f oh) -> p f oh", oh=8),
                s2.rearrange("p (f oh) -> p f oh", oh=8),
                sc[:, None, :].broadcast_to([P, F_SIZE, 8]),
            )
            if t == R:
                # v_sb[o_lo,(f,oh)] -> transpose -> [(f,oh), o_lo] which is
                # exactly T-flat (f*1024+o); host un-transposes for free
                v_bf = small.tile([P, P], BF16, tag="v_bf", bufs=1)
                nc.vector.tensor_copy(v_bf, v_sb)
                v_ts = small.tile([P, P], BF16, tag="v_ts", bufs=1)
                nc.sync.dma_start(v_ts, v_bf, transpose=True)
                nc.sync.dma_start(
                    v_out.rearrange("f (oh ol) -> (f oh) ol", ol=P), v_ts)
            else:
                if t == 1:
                    nc.scalar.copy(w_acc, v_sb)
                else:
                    nc.vector.tensor_add(w_acc, w_acc, v_sb)
                w_bf = small.tile([P, P], BF16, tag="w_bf", bufs=1)
                nc.vector.tensor_copy(w_bf, w_acc)
                w_ts = small.tile([P, P], BF16, tag="w_ts", bufs=1)
                nc.sync.dma_start(w_ts, w_bf, transpose=True)
                w_dram = dram.tile([ROW], BF16, tag="w_dram")
                nc.sync.dma_start(
                    w_dram.rearrange("(a b) -> a b", b=P), w_ts)


def _build(routing_num: int):
    R = int(routing_num)
    assert R >= 1
    nc = bacc.Bacc(
        "TRN2", target_bir_lowering=False, debug=False, num_devices=CORES)
    uh = nc.dram_tensor("uh", [I_LOC, ROW], BF16, kind="ExternalInput")
    v_out = nc.dram_tensor("v_out", [F_SIZE, OUT_NODES], BF16,
                           kind="ExternalOutput")
    rg = [list(range(CORES))]
    with tile.TileContext(nc) as tc:
        _body(nc, tc, uh.ap(), v_out.ap(), R, rg)
    nc.compile()
    return nc


_CACHE: dict = {}


def _get_nc(routing_num: int):
    R = int(routing_num)
    if R not in _CACHE:
        _CACHE[R] = _build(R)
    return _CACHE[R]


def _shard(u_hat: np.ndarray):
    uh = np.asarray(u_hat, dtype=np.float32)
    assert uh.shape == (IN_NODES * OUT_NODES, F_SIZE), uh.shape
    # [i, o, f] -> [i, f, o] ("T" layout, o contiguous innermost), bf16
    uh = uh.reshape(IN_NODES, OUT_NODES, F_SIZE).transpose(0, 2, 1)
    uh = np.ascontiguousarray(uh, dtype=ml_dtypes.bfloat16)
    uh = uh.reshape(IN_NODES, ROW)
    return [{"uh": uh[k * I_LOC:(k + 1) * I_LOC]} for k in range(CORES)]


def run(u_hat, routing_num, trace=False):
    nc = _get_nc(routing_num)
    in_maps = _shard(u_hat)
    res = bass_utils.run_bass_kernel_spmd(
        nc, in_maps, core_ids=list(range(CORES)), trace=trace)
    return res


def kernel(u_hat, routing_num):
    res = run(u_hat, routing_num, trace=False)
    # device returns v in [f, o] (T) layout bf16; un-transpose on host
    v_t = np.asarray(res.results[0]["v_out"], dtype=np.float32)
    return np.ascontiguousarray(v_t.reshape(F_SIZE, OUT_NODES).T)


# revision 34
# speedup vs baseline: 1.0205x; 1.0058x over previous
"""DGL capsule routing layer on 8 trn2 NeuronCores (Bass/Tile).

Math: for routing_num iterations,
    c = softmax(b, axis=out)                        # b0 = 0
    s = einsum('io,iof->of', c, uh)
    v = squash(s)
    b = b + einsum('iof,of->io', uh, v)
Output: final v [OUT, F].

Key identity: b_t = uh . (v_1 + ... + v_{t-1}) (b is linear in uh), so each
iteration is one pass over uh with w_t = cumulative sum of v's:
    pass t: b = sum_f uh[i,o,f]*w[o,f]; e = exp(b); r_i = 1/sum_o e
            s[o,f] = sum_i r_i * e[i,o] * uh[i,o,f]   (partial per core)
            AllReduce(s); v = squash(s); w += v
Pass 1 has c uniform (=1/OUT) so it is a pure PE pass.

Performance structure (vs the f32 streaming baseline):
  - u_hat is cast to bf16 AND transposed to [i, f, o] on the host. With o
    contiguous innermost, every big DVE elementwise op has both operands
    16-bit/step-1/4B-aligned, which is exactly the condition for the DVE's
    2x_1P packed mode (2 elem/cycle/lane) -- f32 or broadcast-stride
    operands would pin them to 1x.
  - 3 of 4 i-blocks are SBUF-resident bf16 (loaded once in pass 1); the 4th
    streams from HBM each pass (DMA is otherwise idle in passes >= 2).
  - b = sum_f uh*w is a contiguous binary tree of TT adds (all 2x) instead
    of a 1x segmented reduce; p = e*uh is issued per-f as fully contiguous
    [P,1024] TTs (2x) instead of a broadcast TT (1x).
  - s-partials for block pairs accumulate in one PSUM tile, halving the
    1-lane PSUM->SBUF flushes; the AllReduce runs in bf16 (2x16384 = 64KiB).
  - GpSimd (slow ucode TT, ~60 G elem/s) takes a minority share of the tm
    muls; ACT does exp + psum flushes; PE does the rank-1 s matmuls.

Data layouts: SBUF/DRAM s/w vectors are (f,o)-flat ("T" layout, o fastest);
the post-AllReduce load and the w_dram store use mirrored strided APs to
convert to/from the o-major [128, (j f)] spread (o = p*8+j) used by squash.
"""

import numpy as np
import ml_dtypes
from contextlib import ExitStack

import concourse.bass as bass
import concourse.mybir as mybir
import concourse.tile as tile
from concourse import bacc
from concourse import bass_utils

F32 = mybir.dt.float32
BF16 = mybir.dt.bfloat16
AX = mybir.AxisListType
AF = mybir.ActivationFunctionType

IN_NODES, OUT_NODES, F_SIZE = 4096, 1024, 16
CORES = 8
I_LOC = IN_NODES // CORES          # 512 in-nodes per core
ROW = OUT_NODES * F_SIZE           # 16384 elems per in-node row
P = 128
NBLK = I_LOC // P                  # 4 i-blocks per core
NRES = 3                           # blocks 0..2 SBUF-resident; block 3 streamed
NPAIR = NBLK // 2                  # 2 block pairs
CH = 4096                          # tm chunk width (4 f-slices x 1024 o)
NCH = ROW // CH                    # 4 tm chunks per block
SEG = 2048                         # psum segment width (f-pair x 1024 o)
NSEG = ROW // SEG                  # 8 segments per block
MMW = 512                          # matmul moving width
O2 = OUT_NODES                     # 1024
GP_TM = set()                      # tm chunks on GpSimd: its ucode TT is ~4x
                                   # slower AND stalls DVE via the shared
                                   # SBUF port -- keep everything on DVE
N_WARM_MM = 24                     # junk matmuls to lift the PE HAM throttle


def _body(nc, tc, uh, v_out, R, rg):
    uh_t = uh.rearrange("(n p) r -> n p r", p=P)   # [NBLK, 128, 16384] bf16

    with ExitStack() as ctx:
        persist = ctx.enter_context(tc.tile_pool(name="persist", bufs=1))
        tmpool = ctx.enter_context(tc.tile_pool(name="tmpool", bufs=2))
        t8pool = ctx.enter_context(tc.tile_pool(name="t8pool", bufs=2))
        ppool = ctx.enter_context(tc.tile_pool(name="ppool", bufs=2))
        spool = ctx.enter_context(tc.tile_pool(name="spool", bufs=2))
        small = ctx.enter_context(tc.tile_pool(name="small", bufs=2))
        pspool = ctx.enter_context(tc.tile_pool(name="pspool", bufs=2, space="PSUM"))
        dram = ctx.enter_context(tc.tile_pool(name="dram", bufs=2, space="DRAM"))

        c0 = persist.tile([P, 1], BF16, name="c0")
        nc.vector.memset(c0, 1.0 / OUT_NODES)
        uh_sb = [persist.tile([P, ROW], BF16, name=f"uh{b}") for b in range(NRES)]
        w_sb = w_acc = None
        if R > 1:
            # one tile per CH chunk so a tm mul only waits on its own
            # chunk's broadcast, not the whole-w tile-level dependency
            w_sb = [persist.tile([P, CH], BF16, name=f"w{c}")
                    for c in range(NCH)]
            w_acc = persist.tile([P, P], BF16, name="w_acc")

        # tiny dummy AllReduce: pays the ~60us first-collective setup cost
        # concurrently with pass 1 instead of on the pass-1 -> pass-2 boundary
        warm_in = dram.tile([16], BF16, tag="warm_in")
        warm_out = dram.tile([16], BF16, tag="warm_out")
        nc.gpsimd.collective_compute(
            "AllReduce", mybir.AluOpType.add, replica_groups=rg,
            ins=[warm_in.opt()], outs=[warm_out.opt()],
        )
        # back-to-back junk matmuls: ~3.4us of sustained PE busy flips the
        # HAM clock gate from 1.2 to 2.4 GHz before the real matmuls start
        junk = persist.tile([P, MMW], BF16, name="junk")
        nc.vector.memset(junk, 0.0)
        ps_warm = pspool.tile([1, SEG], F32, tag="ps")
        for _ in range(N_WARM_MM):
            nc.tensor.matmul(ps_warm[:, :MMW], c0, junk,
                             start=True, stop=True, skip_group_check=True)

        # resident uh upload happens inside the pass-1 segment loop, one
        # [P, SEG] column-slice per block per segment, so each psum group
        # closes as soon as its own ~2 MB lands (sub-tile deps) instead of
        # stalling on whole-block loads

        def stream_chunk(lo, width):
            st = spool.tile([P, width], BF16, tag="st")
            nc.sync.dma_start(st, uh_t[NBLK - 1, :, lo:lo + width])
            return st

        def warm_mm(n=2):
            # keep the PE HAM window busy so phase-B matmuls run at 2.4 GHz
            pw = pspool.tile([1, SEG], F32, tag="ps")
            for _ in range(n):
                nc.tensor.matmul(pw[:, :MMW], c0, junk,
                                 start=True, stop=True, skip_group_check=True)

        w_dram = None
        for t in range(1, R + 1):
            ar_in = dram.tile([1 if t == 1 else NPAIR, ROW], BF16,
                              tag="ar1" if t == 1 else "ar_in")
            if t > 1:
                # broadcast w_T (written at the end of pass t-1) to all
                # 128 partitions, chunk by chunk
                wd_b = w_dram.unsqueeze(0)
                for c in range(NCH):
                    sl = slice(c * CH, (c + 1) * CH)
                    nc.sync.dma_start(
                        w_sb[c], wd_b[:, sl].broadcast_to([P, CH]))
            if t == 1:
                # c is uniform: pure PE pass, all 4 blocks accumulate into
                # one psum tile per segment (half the flushes, half the AR)
                for s in range(NSEG):
                    ss = slice(s * SEG, (s + 1) * SEG)
                    if s % 2 == 0:
                        src2 = stream_chunk(s * SEG, 2 * SEG)
                        # 1 MB-granular resident loads, alternating the two
                        # HWDGE queues (one ring drains serially; two run
                        # in parallel)
                        cs = slice(s * SEG, (s + 2) * SEG)
                        for blk in range(NRES):
                            eng = nc.sync if (s // 2 + blk) % 2 else nc.scalar
                            eng.dma_start(uh_sb[blk][:, cs], uh_t[blk, :, cs])
                    ps = pspool.tile([1, SEG], F32, tag="ps")
                    for blk in range(NBLK):
                        if blk < NRES:
                            src = uh_sb[blk][:, ss]
                        elif s % 2 == 0:
                            src = src2[:, :SEG]
                        else:
                            src = src2[:, SEG:]
                        for cix in range(SEG // MMW):
                            msl = slice(cix * MMW, (cix + 1) * MMW)
                            nc.tensor.matmul(
                                ps[:, msl], c0, src[:, msl],
                                start=(blk == 0), stop=(blk == NBLK - 1),
                                skip_group_check=True,
                            )
                    fl = small.tile([1, SEG], BF16, tag="fl")
                    # DVE is idle in pass 1: alternate flush engines so the
                    # PE is never flush-gated (keeps the HAM clock warm too)
                    if s % 2 == 0:
                        nc.scalar.copy(fl, ps)
                    else:
                        nc.vector.tensor_copy(fl, ps)
                    nc.sync.dma_start(ar_in[0, ss], fl)
            for pair in range(NPAIR if t > 1 else 0):
                es, rinvs = [None, None], [None, None]
                if t > 1:
                    # phase A: b = sum_f uh*w via tm mul + tree adds
                    for j in range(2):
                        blk = 2 * pair + j
                        tms = []
                        for c in range(NCH):
                            sl = slice(c * CH, (c + 1) * CH)
                            if blk < NRES:
                                src = uh_sb[blk][:, sl]
                            else:
                                src = stream_chunk(c * CH, CH)
                            tm = tmpool.tile([P, CH], BF16, tag="tm")
                            eng = nc.gpsimd if c in GP_TM else nc.vector
                            eng.tensor_mul(tm, src, w_sb[c])
                            tms.append(tm)
                        t8a = t8pool.tile([P, CH], BF16, tag="t8")
                        nc.vector.tensor_add(t8a, tms[0], tms[1])
                        t8b = t8pool.tile([P, CH], BF16, tag="t8")
                        nc.vector.tensor_add(t8b, tms[2], tms[3])
                        nc.vector.tensor_add(t8a, t8a, t8b)
                        nc.vector.tensor_add(
                            t8a[:, :SEG], t8a[:, :SEG], t8a[:, SEG:])
                        b_t = small.tile([P, O2], BF16, tag="b")
                        nc.vector.tensor_add(
                            b_t, t8a[:, :O2], t8a[:, O2:SEG])
                        e_t = small.tile([P, O2], BF16, tag="e", bufs=3)
                        den = small.tile([P, 1], F32, tag="den", bufs=4)
                        nc.scalar.activation(e_t, b_t, AF.Exp, accum_out=den)
                        rinv_f = small.tile([P, 1], F32, tag="rinv_f", bufs=4)
                        nc.vector.reciprocal(rinv_f, den)
                        rinv_b = small.tile([P, 1], BF16, tag="rinv", bufs=4)
                        nc.vector.tensor_copy(rinv_b, rinv_f)
                        es[j], rinvs[j] = e_t, rinv_b
                        warm_mm()
                # phase B: p = e*uh per f-slice; psum[1,SEG] += rinv . p
                for s in range(NSEG):
                    ss = slice(s * SEG, (s + 1) * SEG)
                    ps = pspool.tile([1, SEG], F32, tag="ps")
                    for j in range(2):
                        blk = 2 * pair + j
                        if blk < NRES:
                            src = uh_sb[blk][:, ss]
                        elif s % 2 == 0:
                            src2 = stream_chunk(s * SEG, 2 * SEG)
                            src = src2[:, :SEG]
                        else:
                            src = src2[:, SEG:]
                        pp = ppool.tile([P, SEG], BF16, tag="pp")
                        nc.vector.tensor_mul(
                            pp.rearrange("p (g o) -> p g o", g=2),
                            src.rearrange("p (g o) -> p g o", g=2),
                            es[j][:, None, :].broadcast_to([P, 2, O2]))
                        mv, stat = pp, rinvs[j]
                        for cix in range(SEG // MMW):
                            msl = slice(cix * MMW, (cix + 1) * MMW)
                            nc.tensor.matmul(
                                ps[:, msl], stat, mv[:, msl],
                                start=(j == 0), stop=(j == 1),
                                skip_group_check=True,
                            )
                    fl = small.tile([1, SEG], BF16, tag="fl")
                    nc.scalar.copy(fl, ps)
                    nc.sync.dma_start(ar_in[pair, ss], fl)
            ar_out = dram.tile([1 if t == 1 else NPAIR, ROW], BF16,
                               tag="ar1o" if t == 1 else "ar_out")
            nc.gpsimd.collective_compute(
                "AllReduce", mybir.AluOpType.add, replica_groups=rg,
                ins=[ar_in.opt()], outs=[ar_out.opt()],
            )
            # XBAR-transposed load: sld[o_lo, (f,oh)] = s_T[f, oh*128+o_lo]
            slds = []
            for r in range(1 if t == 1 else NPAIR):
                sld = small.tile([P, P], BF16, tag="sld")
                nc.sync.dma_start(
                    sld, ar_out[r].rearrange("(a b) -> a b", b=P),
                    transpose=True)
                slds.append(sld)
            s2 = small.tile([P, P], F32, tag="s2", bufs=1)
            if t == 1:
                nc.vector.tensor_copy(s2, slds[0])
            else:
                nc.vector.tensor_add(s2, slds[0], slds[1])
            # squash: v = s * sqrt(sq)/(1+sq), sq = sum_f s^2
            # layout here is [o_lo, (f, oh)]: f has stride 8, oh stride 1
            ssq = small.tile([P, P], F32, tag="ssq", bufs=1)
            nc.scalar.activation(ssq, s2, AF.Square)
            sq = small.tile([P, 8], F32, tag="sq", bufs=1)
            nc.vector.reduce_sum(
                sq, ssq.rearrange("p (f oh) -> p oh f", oh=8), axis=AX.X)
            y = small.tile([P, 8], F32, tag="y", bufs=1)
            nc.scalar.sqrt(y, sq)
            d1 = small.tile([P, 8], F32, tag="d1", bufs=1)
            nc.vector.tensor_scalar_add(d1, sq, 1.0)
            rd = small.tile([P, 8], F32, tag="rd", bufs=1)
            nc.vector.reciprocal(rd, d1)
            sc = small.tile([P, 8], F32, tag="sc", bufs=1)
            nc.vector.tensor_mul(sc, y, rd)
            v_sb = small.tile([P, P], F32, tag="v_sb", bufs=1)
            nc.vector.tensor_mul(
                v_sb.rearrange("p (f oh) -> p f oh", oh=8),
                s2.rearrange("p (f oh) -> p f oh", oh=8),
                sc[:, None, :].broadcast_to([P, F_SIZE, 8]),
            )
            if t == R:
                # v_sb[o_lo,(f,oh)] -> transpose -> [(f,oh), o_lo] which is
                # exactly T-flat (f*1024+o); host un-transposes for free
                v_bf = small.tile([P, P], BF16, tag="v_bf", bufs=1)
                nc.vector.tensor_copy(v_bf, v_sb)
                v_ts = small.tile([P, P], BF16, tag="v_ts", bufs=1)
                nc.sync.dma_start(v_ts, v_bf, transpose=True)
                nc.sync.dma_start(
                    v_out.rearrange("f (oh ol) -> (f oh) ol", ol=P), v_ts)
            else:
                if t == 1:
                    nc.scalar.copy(w_acc, v_sb)
                else:
                    nc.vector.tensor_add(w_acc, w_acc, v_sb)
                w_ts = small.tile([P, P], BF16, tag="w_ts", bufs=1)
                nc.sync.dma_start(w_ts, w_acc, transpose=True)
                w_dram = dram.tile([ROW], BF16, tag="w_dram")
                nc.sync.dma_start(
                    w_dram.rearrange("(a b) -> a b", b=P), w_ts)


def _build(routing_num: int):
    R = int(routing_num)
    assert R >= 1
    nc = bacc.Bacc(
        "TRN2", target_bir_lowering=False, debug=False, num_devices=CORES)
    uh = nc.dram_tensor("uh", [I_LOC, ROW], BF16, kind="ExternalInput")
    v_out = nc.dram_tensor("v_out", [F_SIZE, OUT_NODES], BF16,
                           kind="ExternalOutput")
    rg = [list(range(CORES))]
    with tile.TileContext(nc) as tc:
        _body(nc, tc, uh.ap(), v_out.ap(), R, rg)
    nc.compile()
    return nc


_CACHE: dict = {}


def _get_nc(routing_num: int):
    R = int(routing_num)
    if R not in _CACHE:
        _CACHE[R] = _build(R)
    return _CACHE[R]


def _shard(u_hat: np.ndarray):
    uh = np.asarray(u_hat, dtype=np.float32)
    assert uh.shape == (IN_NODES * OUT_NODES, F_SIZE), uh.shape
    # [i, o, f] -> [i, f, o] ("T" layout, o contiguous innermost), bf16
    uh = uh.reshape(IN_NODES, OUT_NODES, F_SIZE).transpose(0, 2, 1)
    uh = np.ascontiguousarray(uh, dtype=ml_dtypes.bfloat16)
    uh = uh.reshape(IN_NODES, ROW)
    return [{"uh": uh[k * I_LOC:(k + 1) * I_LOC]} for k in range(CORES)]


def run(u_hat, routing_num, trace=False):
    nc = _get_nc(routing_num)
    in_maps = _shard(u_hat)
    res = bass_utils.run_bass_kernel_spmd(
        nc, in_maps, core_ids=list(range(CORES)), trace=trace)
    return res


def kernel(u_hat, routing_num):
    res = run(u_hat, routing_num, trace=False)
    # device returns v in [f, o] (T) layout bf16; un-transpose on host
    v_t = np.asarray(res.results[0]["v_out"], dtype=np.float32)
    return np.ascontiguousarray(v_t.reshape(F_SIZE, OUT_NODES).T)
